# revision 46
# baseline (speedup 1.0000x reference)
"""Trainium2 Bass kernel for nn_MimicLoss (masked-MSE mimic loss).

Data-parallel over batch: 8 NeuronCores x 4 samples. Per core:
  1. rasterize per-sample union-of-positive-boxes masks from the 8192
     priors directly in priors-on-partitions layout: bounds [128, 64]
     (lane l, chunk c <-> prior l*64+c), coverage indicators via 3 DVE
     tensor ops per axis over broadcast APs (bf16 0/1 outputs), then one
     bf16 matmul per 128-prior chunk accumulates
       hit[h, (b,w)] += sum_p cov_y[p,h] * (cov_x[p,w] * pos[p,b])
     mask = hit > 0 with fused per-sample pixel counts (accum_out).
  2. stream map_s2/map_t2 [4,256,64,64] f32 as 64x 0.5MB tiles on the
     sync HWDGE ring (a single ring sustains ~375 GB/s; a second ring
     only adds SBUF-port contention that stalls the compute engines).
     d2 = bf16((s - t))^2: subtract on DVE (GpSimd is locked out of
     SBUF whenever DVE runs back-to-back, so it only takes the final
     units' chains once DVE quiets), Square on ScalarE; channel
     colsums via bf16 PE matmuls with a 4-column eye stationary into
     [4, 512] PSUM tiles (4 hw-segments per accumulation group).
     Raster prep (cov compares + xb4) is sliced into half-chunk items
     interleaved 2-per-qunit behind the stream subtracts on DVE.
  3. colsums and mask reshaped to [128, 32] via tiny SBUF->SBUF DMAs;
     multiply+reduce per sample on DVE.
Host: gather [1, 8] per core (4 raw contributions + 4 mask sums), apply
divide by (msum * C) and the sequential empty-mask-resets-loss scan, /B.

Self-contained: shapes hardcoded for map_t2/map_s2 [32,256,64,64] f32,
priors [8192,4] f32, mimic_label [32,8192] int32.
"""
import sys

sys.path.insert(0, "/opt/trn_rl_repo")

import ml_dtypes
import numpy as np

import concourse.bacc as bacc
import concourse.bass as bass
import concourse.tile as tile
from concourse import mybir
from concourse.alu_op_type import AluOpType as Op

F32 = mybir.dt.float32
I32 = mybir.dt.int32
BF16 = mybir.dt.bfloat16
AF = mybir.ActivationFunctionType

B, C, H, W = 32, 256, 64, 64
P = 8192
N_CORES = 8
BPC = B // N_CORES          # samples per core
HW = H * W                  # 4096
NCHUNK = P // 128           # 64 prior chunks
QT = HW // 4                # 1024: free size of one stream tile (hw quarter)
QW = 512                    # colsum segment width (PSUM bank)


def build_nc():
    nc = bacc.Bacc("TRN2", debug=False)

    s = nc.dram_tensor("s", [BPC, C, H, W], F32, kind="ExternalInput")
    t = nc.dram_tensor("t", [BPC, C, H, W], F32, kind="ExternalInput")
    priors = nc.dram_tensor("priors", [P, 4], F32, kind="ExternalInput")
    # labels_m[l, c*BPC+b] = mimic_label[b, l*64+c] (host-permuted)
    labels_m = nc.dram_tensor("labels_m", [128, NCHUNK * BPC], I32,
                              kind="ExternalInput")
    iota64 = nc.dram_tensor("iota64", [128, 64], F32, kind="ExternalInput")
    onesq = nc.dram_tensor("onesq", [128, 16], BF16, kind="ExternalInput")
    idn = nc.dram_tensor("idn", [128, 256], F32, kind="ExternalInput")
    out = nc.dram_tensor("out", [1, 2 * BPC], F32, kind="ExternalOutput")

    s_flat = s[:].rearrange("b c h w -> b c (h w)")
    t_flat = t[:].rearrange("b c h w -> b c (h w)")
    priors_ap = priors[:].rearrange("(l c) j -> l c j", l=128)

    with tile.TileContext(nc) as tc:
        with (
            tc.tile_pool(name="const", bufs=1) as constp,
            tc.tile_pool(name="small", bufs=1) as small,
            tc.tile_pool(name="gh", bufs=2) as ghp,
            tc.tile_pool(name="stream_s", bufs=10) as pool_s,
            tc.tile_pool(name="stream_t", bufs=10) as pool_t,
            tc.tile_pool(name="stream_d2", bufs=6) as pool_d2,
            tc.tile_pool(name="segp", bufs=8) as segp,
            tc.tile_pool(name="ps_hit", bufs=1, space="PSUM") as ps_hit,
            tc.tile_pool(name="ps_cs", bufs=3, space="PSUM") as ps_cs,
            tc.tile_pool(name="ps_d", bufs=1, space="PSUM") as ps_d,
            tc.tile_pool(name="ps_out", bufs=1, space="PSUM") as ps_out,
        ):
            # ---- stream loads: [128, 1024] quarter tiles on the sync ring ----
            pending = {}

            def emit_qunit_loads(qu):
                b, hh2 = divmod(qu, 4)
                tiles = []
                for cc in range(2):
                    s_t = pool_s.tile([128, QT], F32)
                    nc.sync.dma_start(
                        s_t[:],
                        s_flat[b, cc * 128:(cc + 1) * 128,
                               hh2 * QT:(hh2 + 1) * QT],
                    )
                    t_t = pool_t.tile([128, QT], F32)
                    nc.sync.dma_start(
                        t_t[:],
                        t_flat[b, cc * 128:(cc + 1) * 128,
                               hh2 * QT:(hh2 + 1) * QT],
                    )
                    tiles.append((s_t, t_t))
                pending[qu] = tiles

            # deep prefetch: first 3 qunits issued before anything else so
            # the stream ring starts immediately
            for qu in range(3):
                emit_qunit_loads(qu)

            # ---- constants & small inputs ----
            priors_sb = small.tile([128, NCHUNK, 4], F32)
            nc.scalar.dma_start(priors_sb[:], priors_ap)
            labels_sb = small.tile([128, NCHUNK, BPC], I32)
            nc.scalar.dma_start(
                labels_sb[:],
                labels_m[:].rearrange("l (c b) -> l c b", b=BPC),
            )
            iota_sb = constp.tile([128, 64], F32)
            nc.scalar.dma_start(iota_sb[:], iota64[:])
            onesq_sb = constp.tile([128, 16], BF16)
            nc.scalar.dma_start(onesq_sb[:], onesq[:])
            idn_sb = constp.tile([128, 256], F32)
            nc.scalar.dma_start(idn_sb[:], idn[:])
            ones_sb = constp.tile([128, 1], F32)
            nc.vector.memset(ones_sb[:], 1.0)

            # pos = (label > 0) as bf16 for the raster multiplies
            pos_f = small.tile([128, NCHUNK, BPC], F32)
            nc.vector.tensor_copy(pos_f[:], labels_sb[:])
            nc.vector.tensor_single_scalar(pos_f[:], pos_f[:], 0.0, Op.is_gt)
            pos_bf = small.tile([128, NCHUNK, BPC], BF16)
            nc.vector.tensor_copy(pos_bf[:], pos_f[:])

            # ---- box bounds minus 1: (c +- d/2)*64 - 1, f32 [128, 64] ----
            cx = priors_sb[:, :, 0]
            cy = priors_sb[:, :, 1]
            bw = priors_sb[:, :, 2]
            bh = priors_sb[:, :, 3]
            hw_half = small.tile([128, NCHUNK], F32)
            hh_half = small.tile([128, NCHUNK], F32)
            nc.vector.tensor_single_scalar(hw_half[:], bw, 0.5, Op.mult)
            nc.vector.tensor_single_scalar(hh_half[:], bh, 0.5, Op.mult)
            xm1 = small.tile([128, NCHUNK], F32)
            xx1 = small.tile([128, NCHUNK], F32)
            ym1 = small.tile([128, NCHUNK], F32)
            yy1 = small.tile([128, NCHUNK], F32)
            nc.vector.tensor_tensor(xm1[:], cx, hw_half[:], Op.subtract)
            nc.vector.tensor_scalar(xm1[:], xm1[:], 64.0, -1.0, Op.mult, Op.add)
            nc.vector.tensor_tensor(xx1[:], cx, hw_half[:], Op.add)
            nc.vector.tensor_scalar(xx1[:], xx1[:], 64.0, -1.0, Op.mult, Op.add)
            nc.vector.tensor_tensor(ym1[:], cy, hh_half[:], Op.subtract)
            nc.vector.tensor_scalar(ym1[:], ym1[:], 64.0, -1.0, Op.mult, Op.add)
            nc.vector.tensor_tensor(yy1[:], cy, hh_half[:], Op.add)
            nc.vector.tensor_scalar(yy1[:], yy1[:], 64.0, -1.0, Op.mult, Op.add)

            # ---- coverage indicators [128(p), chunk, pix] as bf16 0/1:
            # cov = (pix > lo-1) - (pix > hi-1)  (hi >= lo so this is the AND)
            iota_b = iota_sb[:].rearrange("l (o j) -> l o j", o=1).broadcast_to(
                [128, NCHUNK, 64]
            )

            # cov compare/combine ops are emitted in CHUNK-RANGE HALVES so
            # they interleave with the per-qunit stream subtracts.
            covy = small.tile([128, NCHUNK, 64], BF16)
            covx = small.tile([128, NCHUNK, 64], BF16)
            gty = ghp.tile([128, NCHUNK, 64], BF16, tag="gty")
            gtx = ghp.tile([128, NCHUNK, 64], BF16, tag="gtx")
            HC = NCHUNK // 2

            def emit_cov_item(cov, gt, lo, hi, h, stage):
                sl = slice(h * HC, (h + 1) * HC)
                iota_h = iota_sb[:].rearrange(
                    "l (o j) -> l o j", o=1
                ).broadcast_to([128, HC, 64])
                if stage == 0:
                    lo_b = lo[:, sl].rearrange(
                        "l (c o) -> l c o", o=1
                    ).broadcast_to([128, HC, 64])
                    nc.vector.tensor_tensor(cov[:, sl, :], iota_h, lo_b, Op.is_gt)
                elif stage == 1:
                    hi_b = hi[:, sl].rearrange(
                        "l (c o) -> l c o", o=1
                    ).broadcast_to([128, HC, 64])
                    nc.vector.tensor_tensor(gt[:, sl, :], iota_h, hi_b, Op.is_gt)
                else:
                    nc.vector.tensor_tensor(
                        cov[:, sl, :], cov[:, sl, :], gt[:, sl, :], Op.subtract
                    )

            # ---- xb4[p, c, b, w] = covx[p, c, w] * pos[p, c, b], 8 groups ----
            xb4 = small.tile([128, NCHUNK, BPC, 64], BF16)
            GC = NCHUNK // 8

            def emit_xb4_group(g):
                sl = slice(g * GC, (g + 1) * GC)
                covx_b = covx[:, sl, :].rearrange(
                    "l c (o w) -> l c o w", o=1
                ).broadcast_to([128, GC, BPC, 64])
                pos_b = pos_bf[:, sl, :].rearrange(
                    "l c (b o) -> l c b o", o=1
                ).broadcast_to([128, GC, BPC, 64])
                nc.vector.tensor_tensor(xb4[:, sl, :, :], covx_b, pos_b, Op.mult)

            hit = ps_hit.tile([64, BPC * 64], F32)

            def emit_raster_chunk(c):
                nc.tensor.matmul(
                    hit[:],
                    covy[:, c, :],
                    xb4[:, c, :, :].rearrange("l b w -> l (b w)"),
                    start=(c == 0),
                    stop=(c == NCHUNK - 1),
                )

            # mask + fused per-sample pixel counts
            mask_f = small.tile([64, BPC * 64], F32)
            gmat = small.tile([128, 2 * BPC], F32)
            nc.vector.memset(gmat[:], 0.0)

            def emit_mask_and_msums():
                nc.vector.tensor_single_scalar(
                    mask_f[:], hit[:], 0.0, Op.is_gt
                )
                for b in range(BPC):
                    nc.vector.tensor_reduce(
                        gmat[0:64, BPC + b:BPC + b + 1],
                        mask_f[:, b * 64:(b + 1) * 64],
                        mybir.AxisListType.X,
                        Op.add,
                    )

            # ---- per-sample masked dot in [128, 32] layout ----
            def emit_m2(b):
                m2 = small.tile([128, 32], F32, tag=f"m2_{b}")
                nc.sync.dma_start(
                    m2[:],
                    mask_f[:, b * 64:(b + 1) * 64].rearrange(
                        "h (r j) -> h r j", r=2
                    ),
                )
                return m2

            def emit_dot(b, m2, segs):
                cs2 = small.tile([128, 32], F32, tag=f"cs2_{b}")
                for hh in range(2):
                    nc.sync.dma_start(
                        cs2[hh * 64:(hh + 1) * 64, :],
                        segs[hh][:].rearrange("q (r j) -> q r j", j=32),
                    )
                scr = small.tile([128, 32], F32, tag=f"scr_{b}")
                nc.vector.tensor_tensor(scr[:], cs2[:], m2[:], Op.mult)
                nc.vector.tensor_reduce(
                    gmat[:, b:b + 1], scr[:], mybir.AxisListType.X, Op.add
                )

            # ---- interleaved stream + raster emission ----
            # qunit qu = (b, hh2): one hw QUARTER of one sample, both c-blocks
            # ([128, 1024] f32 tiles, 0.5 MB DMAs). Fine granularity keeps the
            # pipeline smooth and the ramp/tail short. Subtract alternates
            # GpSimd (cc0) / DVE (cc1); squares on ScalarE (bf16 out).
            seg_store = {}
            csq_store = {}

            def emit_qunit_compute(qu):
                b, hh2 = divmod(qu, 4)
                hhalf = hh2 // 2
                if hh2 % 2 == 0:
                    csq_store[(b, hhalf)] = ps_cs.tile(
                        [BPC, QW], F32, name="csq", tag="csq"
                    )
                csq = csq_store[(b, hhalf)]
                for cc in range(2):
                    s_t, t_t = pending[qu][cc]
                    d2_t = pool_d2.tile([128, QT], BF16)
                    if qu >= 14 and cc == 0:
                        # tail: GpSimd is free once DVE raster work is done;
                        # parallelize the last qunits' chains across engines
                        nc.gpsimd.tensor_tensor(
                            d2_t[:], s_t[:], t_t[:], Op.subtract
                        )
                        nc.scalar.activation(d2_t[:], d2_t[:], AF.Square)
                    elif qu >= 14:
                        nc.vector.tensor_tensor(
                            d2_t[:], s_t[:], t_t[:], Op.subtract
                        )
                        nc.vector.tensor_tensor(
                            d2_t[:], d2_t[:], d2_t[:], Op.mult
                        )
                    elif 2 <= qu <= 9 and cc == 0:
                        # PE-assisted subtract: d = I.s + (-I).t into PSUM,
                        # ScalarE squares straight from PSUM (it sits closer
                        # to PSUM anyway); frees DVE cycles for raster prep
                        d_ps = ps_d.tile([128, QT], F32, name="d_ps", tag="dps")
                        for sg in range(2):
                            sl = slice(sg * QW, (sg + 1) * QW)
                            nc.tensor.matmul(
                                d_ps[:, sl], idn_sb[:, 0:128], s_t[:, sl],
                                start=True, stop=False,
                            )
                            nc.tensor.matmul(
                                d_ps[:, sl], idn_sb[:, 128:256], t_t[:, sl],
                                start=False, stop=True,
                            )
                        nc.scalar.activation(d2_t[:], d_ps[:], AF.Square)
                    else:
                        nc.vector.tensor_tensor(
                            d2_t[:], s_t[:], t_t[:], Op.subtract
                        )
                        nc.scalar.activation(d2_t[:], d2_t[:], AF.Square)
                    for qq in range(2):
                        r = 2 * (hh2 % 2) + qq
                        nc.tensor.matmul(
                            csq[:],
                            onesq_sb[:, 4 * r:4 * r + 4],
                            d2_t[:, qq * QW:(qq + 1) * QW],
                            start=(hh2 % 2 == 0 and cc == 0 and qq == 0),
                            stop=(hh2 % 2 == 1 and cc == 1 and qq == 1),
                        )
                if hh2 % 2 == 1:
                    seg = segp.tile([BPC, QW], F32, tag="seg")
                    nc.scalar.copy(seg[:], csq[:])
                    seg_store[(b, hhalf)] = seg

            # Raster prep schedule: 2 DVE items per qunit for qu 0-9 (cov
            # halves + even xb4 groups), odd xb4 groups on GpSimd; raster
            # matmul groups (8 chunks) emitted in chunk order as soon as
            # their covy half + xb4 group exist. Mask/dots go late, after
            # the raster is finished (~60% through the stream).
            # item = ("cov", cov, gt, lo, hi, h, stage) | ("xb4", g)
            dve_items = {
                0: [("covx", 0, 0), ("covx", 0, 1)],
                1: [("covx", 0, 2), ("covy", 0, 0)],
                2: [("covy", 0, 1), ("covy", 0, 2)],
                3: [("xb4", 0), ("xb4", 1)],
                4: [("xb4", 2), ("xb4", 3)],
                5: [("covx", 1, 0), ("covx", 1, 1)],
                6: [("covx", 1, 2), ("covy", 1, 0)],
                7: [("covy", 1, 1), ("covy", 1, 2)],
                8: [("xb4", 4), ("xb4", 5)],
                9: [("xb4", 6), ("xb4", 7)],
            }
            gp_items = {}
            cov_args = {
                "covy": (covy, gty, ym1, yy1),
                "covx": (covx, gtx, xm1, xx1),
            }
            xb4_done = set()
            covy_halves = set()
            raster_emitted = [0]  # next chunk group to emit (8 chunks each)

            def emit_ready_raster():
                while raster_emitted[0] < 8:
                    r = raster_emitted[0]
                    if r not in xb4_done or (r // 4) not in covy_halves:
                        return
                    for c in range(8 * r, 8 * r + 8):
                        emit_raster_chunk(c)
                    raster_emitted[0] += 1

            NQU = 4 * BPC  # 16 qunits
            for qu in range(NQU):
                b, hh2 = divmod(qu, 4)
                if qu + 3 < NQU:
                    emit_qunit_loads(qu + 3)
                emit_qunit_compute(qu)
                for it in dve_items.get(qu, ()):
                    if it[0] == "xb4":
                        emit_xb4_group(it[1])
                        xb4_done.add(it[1])
                    else:
                        cov, gt, lo, hi = cov_args[it[0]]
                        emit_cov_item(cov, gt, lo, hi, it[1], it[2])
                        if it[0] == "covy" and it[2] == 2:
                            covy_halves.add(it[1])
                for g in gp_items.get(qu, ()):
                    emit_xb4_group(g)
                    xb4_done.add(g)
                emit_ready_raster()
                if qu == 10:
                    emit_mask_and_msums()
                    m2s = [emit_m2(bb) for bb in range(BPC)]
                if qu == 11:
                    emit_dot(0, m2s[0], (seg_store[(0, 0)], seg_store[(0, 1)]))
                    emit_dot(1, m2s[1], (seg_store[(1, 0)], seg_store[(1, 1)]))
                if qu == 12:
                    emit_dot(2, m2s[2], (seg_store[(2, 0)], seg_store[(2, 1)]))
                if qu == 13:
                    # sample 3 dot, H0 half: its colsums just landed; only
                    # the H1 half-chain remains in the kernel tail
                    cs2_3 = small.tile([128, 32], F32, tag="cs2_3")
                    scr3 = small.tile([128, 32], F32, tag="scr3")
                    nc.sync.dma_start(
                        cs2_3[0:64, :],
                        seg_store[(3, 0)][:].rearrange(
                            "q (r j) -> q r j", j=32
                        ),
                    )
                    nc.vector.tensor_tensor(
                        scr3[0:64, :], cs2_3[0:64, :], m2s[3][0:64, :],
                        Op.mult,
                    )
                    nc.vector.tensor_reduce(
                        gmat[0:64, 3:4], scr3[0:64, :],
                        mybir.AxisListType.X, Op.add,
                    )
            # ---- final partition reduce, split: columns != 3 are complete
            # by qu13, so reduce and ship them early; only sample 3's
            # contrib column stays in the kernel tail ----
            gout = ps_out.tile([1, 2 * BPC], F32)
            out_sb = small.tile([1, 2 * BPC], F32)
            nc.tensor.matmul(
                gout[:, 0:3], ones_sb[:], gmat[:, 0:3], start=True, stop=True
            )
            nc.tensor.matmul(
                gout[:, 4:8], ones_sb[:], gmat[:, 4:8], start=True, stop=True
            )
            nc.scalar.copy(out_sb[:, 0:3], gout[:, 0:3])
            nc.scalar.copy(out_sb[:, 4:8], gout[:, 4:8])
            nc.scalar.dma_start(out[0:1, 0:3], out_sb[:, 0:3])
            nc.scalar.dma_start(out[0:1, 4:8], out_sb[:, 4:8])

            nc.sync.dma_start(
                cs2_3[64:128, :],
                seg_store[(3, 1)][:].rearrange("q (r j) -> q r j", j=32),
            )
            nc.vector.tensor_tensor(
                scr3[64:128, :], cs2_3[64:128, :], m2s[3][64:128, :], Op.mult
            )
            nc.vector.tensor_reduce(
                gmat[64:128, 3:4], scr3[64:128, :],
                mybir.AxisListType.X, Op.add,
            )
            nc.tensor.matmul(
                gout[:, 3:4], ones_sb[:], gmat[:, 3:4], start=True, stop=True
            )
            nc.scalar.copy(out_sb[:, 3:4], gout[:, 3:4])
            nc.scalar.dma_start(out[0:1, 3:4], out_sb[:, 3:4])

    nc.compile()
    return nc


_NC_CACHE = {}


def _get_nc():
    if "nc" not in _NC_CACHE:
        _NC_CACHE["nc"] = build_nc()
    return _NC_CACHE["nc"]


def make_in_maps(map_t2, map_s2, priors, mimic_label):
    iota64 = np.broadcast_to(
        np.arange(64, dtype=np.float32)[None, :], (128, 64)
    ).copy()
    onesq = np.zeros((128, 16), dtype=np.float32)
    for q in range(4):
        onesq[:, 4 * q + q] = 1.0
    onesq = onesq.astype(ml_dtypes.bfloat16)
    idn = np.concatenate(
        [np.eye(128, dtype=np.float32), -np.eye(128, dtype=np.float32)], axis=1
    )
    in_maps = []
    for ci in range(N_CORES):
        sl = slice(ci * BPC, (ci + 1) * BPC)
        lab = np.asarray(mimic_label[sl]).astype(np.int32)  # [BPC, P]
        # labels_m[l, c*BPC+b] = lab[b, l*64+c]
        labels_m = np.ascontiguousarray(
            lab.T.reshape(128, NCHUNK, BPC).reshape(128, NCHUNK * BPC)
        )
        in_maps.append(
            {
                "s": np.ascontiguousarray(map_s2[sl]).astype(np.float32),
                "t": np.ascontiguousarray(map_t2[sl]).astype(np.float32),
                "priors": np.ascontiguousarray(priors).astype(np.float32),
                "labels_m": labels_m,
                "iota64": iota64,
                "onesq": onesq,
                "idn": idn,
            }
        )
    return in_maps


def finish_host(core_outs):
    """core_outs: list of [1, 2*BPC] arrays -> scalar loss (float32)."""
    contribs = np.empty(B, np.float64)
    msums = np.empty(B, np.float64)
    for ci in range(N_CORES):
        o = np.asarray(core_outs[ci], dtype=np.float64)
        contribs[ci * BPC:(ci + 1) * BPC] = o[0, :BPC]
        msums[ci * BPC:(ci + 1) * BPC] = o[0, BPC:]
    loss = 0.0
    for i in range(B):
        if msums[i] == 0.0:
            loss = 0.0
        else:
            loss = loss + contribs[i] / msums[i] / C
    return np.float32(loss / B)


def kernel(map_t2, map_s2, priors, mimic_label):
    from concourse.bass_utils import run_bass_kernel_spmd

    nc = _get_nc()
    in_maps = make_in_maps(map_t2, map_s2, priors, mimic_label)
    res = run_bass_kernel_spmd(
        nc, in_maps, core_ids=list(range(N_CORES))
    )
    outs = [res.results[ci]["out"] for ci in range(N_CORES)]
    return finish_host(outs)
